# revision 9
# baseline (speedup 1.0000x reference)
"""Trainium2 Bass kernel for Mistral-style GQA attention (8-core head-parallel).

Sharding: tensor-parallel over heads. Each of the 8 cores owns 4 query
heads + their shared KV head (GQA group), computes q/k/v projections,
RoPE, causal attention and its slice of the o_proj contraction, then the
host sums the 8 partial outputs (the all-reduce of the sharding hint,
done on the host since each partial is written once and overlaps with
compute).

Layout strategy: everything feature-major ([d, t]) so the PE contraction
dim always sits on partitions and no on-chip transposes of activations
are needed:
  - host passes hidden^T and pre-transposed weights,
  - projections produce qT/kT ([d, t]) directly,
  - scores are computed transposed (S^T[j, i]) so the PV matmul consumes
    exp(S^T) directly with V in natural [token, d] layout (V is produced
    feature-major too, then flipped with cheap PE transposes),
  - softmax row sums come from an extra ones-vector matmul; the row max
    is replaced by a constant shift (scores of this problem are bounded
    |s| < ~30, and softmax is shift-invariant as long as exp neither
    overflows nor fully underflows, so exp(s - 25) is exact).
  - RoPE's rotate-half is a partition rotation, which no lane-aligned
    engine can do; instead the projection epilogue writes q*cos and
    q*sin_signed and a SBUF->SBUF DMA performs the rotation, followed by
    one add.

All matmuls run as float32r (full fp32 operands, reduced-precision
multiply): 1 PE cycle/row at moving-dim >= 256, 4x faster than fp32 with
~tf32 accuracy.
"""

import numpy as np

import concourse.bass as bass
import concourse.tile as tile
from concourse import mybir
from concourse.bass_utils import run_bass_kernel_spmd
from concourse.masks import make_identity

F32 = mybir.dt.float32
F32R = mybir.dt.float32r
N_CORES = 8
D = 128          # head dim
QH = 4           # query heads per core
QF = QH * D      # 512 local q features
EXP_SHIFT = 25.0
NEG = -1.0e30

CFG_FULL = dict(B=2, S=2048, H=4096)


def r(ap):
    return ap.bitcast(F32R)


# ---------------------------------------------------------------- program

def build_program(cfg):
    B, S, H = cfg["B"], cfg["S"], cfg["H"]
    T = B * S
    HC = H // 128          # contraction chunks for projections
    TT = 256               # phase-1 token tile
    NT = T // TT
    IT = 512               # phase-2 query tile
    NIT = S // IT
    NJB_MAX = S // 128

    nc = bass.Bass("TRN2", target_bir_lowering=False, debug=False,
                   num_devices=N_CORES)

    xT = nc.dram_tensor("xT", [H, T], F32R, kind="ExternalInput").ap()
    wqT = nc.dram_tensor("wqT", [H, QF], F32R, kind="ExternalInput").ap()
    wkT = nc.dram_tensor("wkT", [H, D], F32R, kind="ExternalInput").ap()
    wvT = nc.dram_tensor("wvT", [H, D], F32R, kind="ExternalInput").ap()
    woT = nc.dram_tensor("woT", [QF, H], F32R, kind="ExternalInput").ap()
    cosq = nc.dram_tensor("cosq", [D, T], F32, kind="ExternalInput").ap()
    sinq = nc.dram_tensor("sinq", [D, T], F32, kind="ExternalInput").ap()
    cosk = nc.dram_tensor("cosk", [D, T], F32, kind="ExternalInput").ap()
    sink = nc.dram_tensor("sink", [D, T], F32, kind="ExternalInput").ap()
    tri = nc.dram_tensor("tri", [128, 128], F32, kind="ExternalInput").ap()
    onesin = nc.dram_tensor("onesin", [128, 128], F32R, kind="ExternalInput").ap()
    opart = nc.dram_tensor("opart", [T, H], F32, kind="ExternalOutput").ap()

    qT_d = nc.dram_tensor("qT_d", [QF, T], F32R).ap()
    kT_d = nc.dram_tensor("kT_d", [D, T], F32R).ap()
    v_d = nc.dram_tensor("v_d", [T, D], F32R).ap()

    with tile.TileContext(nc) as tc:
        # ---------------- constants
        with tc.tile_pool(name="consts", bufs=1) as consts:
            tri_sb = consts.tile([128, 128], F32)
            nc.sync.dma_start(tri_sb[:], tri[:])
            ones_col = consts.tile([128, 1], F32R)
            nc.sync.dma_start(ones_col[:], onesin[:, 0:1])
            ones_row = consts.tile([1, 128], F32R)
            nc.sync.dma_start(ones_row[:], onesin[0:1, :])
            neg_shift = consts.tile([128, 1], F32)
            nc.vector.memset(neg_shift[:], -EXP_SHIFT)

            # ---------------- phase 1: QKV projections + RoPE epilogue
            with tc.tile_pool(name="wq_sb", bufs=HC) as wq_pool, \
                 tc.tile_pool(name="wk_sb", bufs=HC) as wk_pool, \
                 tc.tile_pool(name="wv_sb", bufs=HC) as wv_pool, \
                 tc.tile_pool(name="ident", bufs=1) as ident_pool, \
                 tc.tile_pool(name="x_sb", bufs=min(NT, 2) * HC - HC // 2) as x_pool, \
                 tc.tile_pool(name="cs_sb", bufs=3) as cs_pool, \
                 tc.tile_pool(name="rope", bufs=3) as rope_pool, \
                 tc.tile_pool(name="stage1", bufs=4) as st1_pool, \
                 tc.tile_pool(name="ps1", bufs=6, space="PSUM") as ps1, \
                 tc.tile_pool(name="ps1v", bufs=2, space="PSUM") as ps1v:

                ident = ident_pool.tile([128, 128], F32)
                make_identity(nc, ident[:])

                wq_t = []
                wk_t = []
                wv_t = []
                for hc in range(HC):
                    wt = wq_pool.tile([128, QF], F32R, tag="wq")
                    nc.sync.dma_start(wt[:], wqT[hc * 128:(hc + 1) * 128, :])
                    wq_t.append(wt)
                    kt = wk_pool.tile([128, D], F32R, tag="wk")
                    nc.sync.dma_start(kt[:], wkT[hc * 128:(hc + 1) * 128, :])
                    wk_t.append(kt)
                    vt = wv_pool.tile([128, D], F32R, tag="wv")
                    nc.sync.dma_start(vt[:], wvT[hc * 128:(hc + 1) * 128, :])
                    wv_t.append(vt)

                def rope_store(ps, cos_t, sin_t, dst, rows, t0):
                    """dst[rows, t0:t0+TT] = ps*cos + rot128(ps*sin_signed)."""
                    c_t = rope_pool.tile([128, TT], F32, tag="ropec")
                    nc.vector.tensor_mul(c_t[:], ps[:], cos_t[:])
                    s_t = rope_pool.tile([128, TT], F32, tag="ropes")
                    nc.vector.tensor_mul(s_t[:], ps[:], sin_t[:])
                    sr_t = rope_pool.tile([128, TT], F32, tag="roper")
                    nc.sync.dma_start(sr_t[0:64, :], s_t[64:128, :])
                    nc.sync.dma_start(sr_t[64:128, :], s_t[0:64, :])
                    o_t = rope_pool.tile([128, TT], F32R, tag="ropeo")
                    nc.vector.tensor_add(o_t[:], c_t[:], sr_t[:])
                    nc.gpsimd.dma_start(dst[rows[0]:rows[1], t0:t0 + TT], o_t[:])

                for tt in range(NT):
                    t0 = tt * TT
                    x_t = []
                    for hc in range(HC):
                        xt_ = x_pool.tile([128, TT], F32R, tag="x")
                        nc.sync.dma_start(
                            xt_[:], xT[hc * 128:(hc + 1) * 128, t0:t0 + TT])
                        x_t.append(xt_)
                    cq_t = cs_pool.tile([128, TT], F32, tag="cq")
                    nc.sync.dma_start(cq_t[:], cosq[:, t0:t0 + TT])
                    sq_t = cs_pool.tile([128, TT], F32, tag="sq")
                    nc.sync.dma_start(sq_t[:], sinq[:, t0:t0 + TT])
                    ck_t = cs_pool.tile([128, TT], F32, tag="ck")
                    nc.sync.dma_start(ck_t[:], cosk[:, t0:t0 + TT])
                    sk_t = cs_pool.tile([128, TT], F32, tag="sk")
                    nc.sync.dma_start(sk_t[:], sink[:, t0:t0 + TT])

                    for oq in range(QH):
                        ps_q = ps1.tile([128, TT], F32, tag="psqkv")
                        for hc in range(HC):
                            nc.tensor.matmul(
                                ps_q[:],
                                r(wq_t[hc][:, oq * 128:(oq + 1) * 128]),
                                r(x_t[hc][:]),
                                start=(hc == 0), stop=(hc == HC - 1))
                        rope_store(ps_q, cq_t, sq_t, qT_d,
                                   (oq * 128, (oq + 1) * 128), t0)

                    ps_k = ps1.tile([128, TT], F32, tag="psqkv")
                    for hc in range(HC):
                        nc.tensor.matmul(ps_k[:], r(wk_t[hc][:]), r(x_t[hc][:]),
                                         start=(hc == 0), stop=(hc == HC - 1))
                    rope_store(ps_k, ck_t, sk_t, kT_d, (0, D), t0)

                    ps_v = ps1.tile([128, TT], F32, tag="psqkv")
                    for hc in range(HC):
                        nc.tensor.matmul(ps_v[:], r(wv_t[hc][:]), r(x_t[hc][:]),
                                         start=(hc == 0), stop=(hc == HC - 1))
                    vT_sb = st1_pool.tile([128, TT], F32, tag="vT")
                    nc.vector.tensor_copy(vT_sb[:], ps_v[:])
                    for k2 in range(TT // 128):
                        ps_vt = ps1v.tile([128, 128], F32, tag="psvt")
                        nc.tensor.transpose(ps_vt[:], vT_sb[:, k2 * 128:(k2 + 1) * 128],
                                            ident[:])
                        v_sb = st1_pool.tile([128, 128], F32R, tag="vn")
                        nc.vector.tensor_copy(v_sb[:], ps_vt[:])
                        nc.gpsimd.dma_start(
                            v_d[t0 + k2 * 128:t0 + (k2 + 1) * 128, :], v_sb[:])

            # ---------------- phase 2: attention + o_proj partial
            with tc.tile_pool(name="wo_sb", bufs=QH) as wo_pool, \
                 tc.tile_pool(name="kT", bufs=1) as kT_pool, \
                 tc.tile_pool(name="v_sb2", bufs=T // 128) as v_pool, \
                 tc.tile_pool(name="qt", bufs=4) as qt_pool, \
                 tc.tile_pool(name="pexp", bufs=6) as pexp_pool, \
                 tc.tile_pool(name="attn_sb", bufs=QH * 2) as attn_pool, \
                 tc.tile_pool(name="small", bufs=4) as small_pool, \
                 tc.tile_pool(name="bcast", bufs=4) as bcast_pool, \
                 tc.tile_pool(name="ostage", bufs=8) as out_pool, \
                 tc.tile_pool(name="ps_s", bufs=3, space="PSUM") as ps_s_pool, \
                 tc.tile_pool(name="ps_a", bufs=2, space="PSUM") as ps_a_pool, \
                 tc.tile_pool(name="ps_n", bufs=1, space="PSUM") as ps_n_pool, \
                 tc.tile_pool(name="ps_o", bufs=2, space="PSUM") as ps_o_pool:

                wo_t = []
                for h in range(QH):
                    wt = wo_pool.tile([128, H], F32R, tag="wo")
                    nc.sync.dma_start(wt[:], woT[h * 128:(h + 1) * 128, :])
                    wo_t.append(wt)

                kT_sb = kT_pool.tile([D, T], F32R)
                nc.sync.dma_start(kT_sb[:], kT_d[:])
                v_t = []
                for j in range(T // 128):
                    vt_ = v_pool.tile([128, D], F32R, tag="v")
                    nc.sync.dma_start(vt_[:], v_d[j * 128:(j + 1) * 128, :])
                    v_t.append(vt_)

                for b in range(B):
                    for it in range(NIT):
                        i0 = b * S + it * IT
                        heads = []
                        for h in range(QH):
                            qt_ = qt_pool.tile([128, IT], F32R, tag="q")
                            nc.sync.dma_start(
                                qt_[:], qT_d[h * 128:(h + 1) * 128, i0:i0 + IT])
                            ps_attn = ps_a_pool.tile([128, IT], F32, tag="attn")
                            ps_sums = ps_n_pool.tile([1, IT], F32, tag="sums")
                            njb = (it + 1) * (IT // 128)
                            for jb in range(njb):
                                off = max(0, jb * 128 - it * IT)
                                j0 = b * S + jb * 128
                                ps_sc = ps_s_pool.tile([128, IT], F32, tag="sc")
                                nc.tensor.matmul(
                                    ps_sc[:, off:IT],
                                    r(kT_sb[:, j0:j0 + 128]),
                                    r(qt_[:, off:IT]),
                                    start=True, stop=True)
                                if jb >= it * (IT // 128):
                                    nc.vector.tensor_add(
                                        ps_sc[:, off:off + 128],
                                        ps_sc[:, off:off + 128], tri_sb[:])
                                pexp = pexp_pool.tile([128, IT], F32R, tag="pe")
                                nc.scalar.activation(
                                    pexp[:, off:IT], ps_sc[:, off:IT],
                                    mybir.ActivationFunctionType.Exp,
                                    bias=neg_shift[:])
                                nc.tensor.matmul(
                                    ps_attn[:, off:IT],
                                    r(v_t[(b * S) // 128 + jb][:]),
                                    r(pexp[:, off:IT]),
                                    start=(jb == 0), stop=(jb == njb - 1))
                                nc.tensor.matmul(
                                    ps_sums[0:1, off:IT],
                                    r(ones_col[:]),
                                    r(pexp[:, off:IT]),
                                    start=(jb == 0), stop=(jb == njb - 1))
                            rsb = small_pool.tile([1, IT], F32R, tag="recip")
                            with nc.allow_low_precision(
                                    reason="f32r-rounded reciprocal of softmax sums"):
                                nc.vector.reciprocal(rsb[:], ps_sums[:])
                            ps_b = ps_s_pool.tile([128, IT], F32, tag="sc")
                            nc.tensor.matmul(ps_b[:], r(ones_row[:]), r(rsb[:]),
                                             start=True, stop=True)
                            bsb = bcast_pool.tile([128, IT], F32, tag="bc")
                            nc.scalar.activation(
                                bsb[:], ps_b[:],
                                mybir.ActivationFunctionType.Copy)
                            at_sb = attn_pool.tile([128, IT], F32R, tag="at")
                            nc.vector.tensor_mul(at_sb[:], ps_attn[:], bsb[:])
                            heads.append(at_sb)

                        for st in range(IT // 128):
                            row0 = i0 + st * 128
                            for mt in range(H // 512):
                                ps_o = ps_o_pool.tile([128, 512], F32, tag="o")
                                for h2 in range(QH):
                                    nc.tensor.matmul(
                                        ps_o[:],
                                        r(heads[h2][:, st * 128:(st + 1) * 128]),
                                        r(wo_t[h2][:, mt * 512:(mt + 1) * 512]),
                                        start=(h2 == 0), stop=(h2 == QH - 1))
                                osb = out_pool.tile([128, 512], F32, tag="ost")
                                nc.vector.tensor_copy(osb[:], ps_o[:])
                                nc.gpsimd.dma_start(
                                    opart[row0:row0 + 128, mt * 512:(mt + 1) * 512],
                                    osb[:])

    _split_multi_waits(nc)
    return nc


# ------------------------------------------------- multi-wait legalization

def _split_multi_waits(nc, cap_regular=1, cap_es=2):
    """This container's walrus enforces the HW wait-slot limits (1 sync wait
    per regular instruction, 2 per EventSemaphore); Tile can attach more.
    Engines run their stream in order, so excess waits are hoisted into
    wait-only EventSemaphore instructions immediately before the owner."""
    from bass_rust import SyncInfo

    n = 0
    for f in nc.m.functions:
        for blk in f.blocks:
            out = []
            changed = False
            for inst in blk.instructions:
                si = inst.sync_info
                waits = list(si.on_wait) if (si and si.on_wait) else []
                cap = (cap_es if isinstance(inst, mybir.InstEventSemaphore)
                       else cap_regular)
                if len(waits) > cap:
                    changed = True
                    n += 1
                    keep = waits[-cap:] if cap else []
                    extra = waits[:len(waits) - cap]
                    i = 0
                    while i < len(extra):
                        chunk = extra[i:i + cap_es]
                        es = mybir.InstEventSemaphore(
                            name=f"{inst.name}-wsplit{i}", ins=[], outs=[])
                        es.engine = inst.engine
                        es.sync_info = SyncInfo(on_wait=chunk, on_update=[])
                        out.append(es)
                        i += len(chunk)
                    inst.sync_info = SyncInfo(
                        on_wait=keep,
                        on_update=list(si.on_update) if si.on_update else [])
                out.append(inst)
            if changed:
                try:
                    blk.instructions = out
                except Exception:
                    blk.instructions.clear()
                    blk.instructions.extend(out)
    return n


# ---------------------------------------------------------------- host side

def host_prep(cfg, hidden_states, cos, sin, wq, wk, wv, wo):
    B, S, H = cfg["B"], cfg["S"], cfg["H"]
    T = B * S
    f32 = np.float32

    xT = np.ascontiguousarray(
        hidden_states.reshape(T, H).T).astype(f32, copy=False)
    cos_t = cos.reshape(T, D).T  # [D, T]
    sin_t = sin.reshape(T, D).T
    sign = np.concatenate([np.ones(64, f32), -np.ones(64, f32)])[:, None]
    scale = np.float32(D ** -0.5)
    cosq = np.ascontiguousarray(cos_t * scale).astype(f32, copy=False)
    sinq = np.ascontiguousarray(sin_t * sign * scale).astype(f32, copy=False)
    cosk = np.ascontiguousarray(cos_t).astype(f32, copy=False)
    sink = np.ascontiguousarray(sin_t * sign).astype(f32, copy=False)
    ii = np.arange(128)
    tri = np.where(ii[None, :] >= ii[:, None], 0.0, NEG).astype(f32)

    in_maps = []
    for c in range(N_CORES):
        in_maps.append({
            "xT": xT,
            "wqT": np.ascontiguousarray(wq[c * QF:(c + 1) * QF, :].T),
            "wkT": np.ascontiguousarray(wk[c * D:(c + 1) * D, :].T),
            "wvT": np.ascontiguousarray(wv[c * D:(c + 1) * D, :].T),
            "woT": np.ascontiguousarray(wo[:, c * QF:(c + 1) * QF].T),
            "cosq": cosq, "sinq": sinq, "cosk": cosk, "sink": sink,
            "tri": tri, "onesin": np.ones((128, 128), f32),
        })
    return in_maps


def assemble(cfg, results):
    B, S, H = cfg["B"], cfg["S"], cfg["H"]
    out = results[0]["opart"].astype(np.float32, copy=True)
    for c in range(1, N_CORES):
        out += results[c]["opart"]
    return out.reshape(B, S, H)


def run(cfg, inputs, trace=False, **kwargs):
    nc = build_program(cfg)
    in_maps = host_prep(cfg, **{k: np.asarray(v) for k, v in inputs.items()})
    res = run_bass_kernel_spmd(nc, in_maps, core_ids=list(range(N_CORES)),
                               trace=trace, **kwargs)
    return assemble(cfg, res.results), res


def kernel(**inputs):
    out, _ = run(CFG_FULL, inputs, trace=False)
    return out
